# revision 18
# baseline (speedup 1.0000x reference)
"""GNN message-passing kernel for TRN2, 8-core SPMD (self-contained).

v3 design:
- Node rows sharded 8 ways (NS=N/8 per core), edge rows too (ES=E/8).
- Gathers move ONLY real neighbor rows (packed index streams built on the
  host) from fp8(e4m3) replicas of the node/edge tables. Each 128-row
  gather tile is segment-summed into the destination block's PSUM via a
  matmul with a 0/1 selection matrix M (M[p,d] = 1 iff packed row p
  belongs to dest node d), built on-chip with one DVE is_equal op from a
  per-tile bf16 segment-id column. Pad rows use index 0 with seg=-1 so
  their M column is all zero.
- The mean's 1/cnt scale fuses into the PSUM->SBUF copy (scalar engine
  activation with per-partition scale). Own-node rows load straight into
  the transposed linear input via HWDGE DMA-transpose (bf16). Linear
  layers run in bf16.
- Activations are staged twice per hop: bf16 shard (next hop's own rows)
  and fp8 shard that is AllGathered into the gather tables. Tables use a
  chunk-permuted row layout so chunked AllGathers (issued every few
  blocks) write contiguous ranges and overlap compute.
- Per-block-slot stream tile counts = max over the 8 cores so one SPMD
  program fits every core; compile is specialized per input (cached).
"""
import os as _os
import sys

sys.path.insert(0, '/opt/trn_rl_repo')

import numpy as np
import concourse.bass as bass
import concourse.mybir as mybir
from concourse import tile
from concourse.bacc import Bacc
from concourse.masks import make_identity

F32 = mybir.dt.float32
BF16 = mybir.dt.bfloat16
F8 = mybir.dt.float8e4
I16 = mybir.dt.int16
P = 128


class Cfg:
    def __init__(self, N=8192, E=32768, D=512, DEG=16, DEP=8, K=3, CORES=8):
        self.N, self.E, self.D = N, E, D
        self.DEG, self.DEP, self.K, self.CORES = DEG, DEP, K, CORES
        self.NS = N // CORES
        self.ES = E // CORES
        self.NB = self.NS // P
        self.EB = self.ES // P
        self.DC = D // P          # feature chunks per D
        self.KCN = (2 * D) // P   # contraction chunks, node linear
        self.KCE = (3 * D) // P   # contraction chunks, edge linear
        # AllGather chunk counts. Shared-DRAM collective outputs only allow
        # a single writer instruction, so chunked AllGathers are off.
        self.NCH_N = 1
        self.NCH_E = 1
        assert self.NS % P == 0 and self.ES % P == 0 and D % P == 0
        assert N <= 32768 and E <= 32768  # int16 dma_gather indices


def build(cfg: Cfg, tiles, nt8=True, et8=True):
    """tiles: dict stream -> list of per-block tile counts.
    streams: fa (fw adj), fe (fw eid), ba, be, df (edge fw dep), db."""
    N, E, D = cfg.N, cfg.E, cfg.D
    K, CORES = cfg.K, cfg.CORES
    NS, ES, NB, EB = cfg.NS, cfg.ES, cfg.NB, cfg.EB
    DC, KCN, KCE = cfg.DC, cfg.KCN, cfg.KCE
    NCH_N, NCH_E = cfg.NCH_N, cfg.NCH_E
    NDT = F8 if nt8 else BF16
    EDT = F8 if et8 else BF16

    def offs(ts):
        o = [0]
        for t in ts:
            o.append(o[-1] + t)
        return o

    off = {s: offs(ts) for s, ts in tiles.items()}

    nc = Bacc("TRN2", target_bir_lowering=False, debug=False, num_devices=CORES,
              num_swdge_queues=4)

    # ---- external inputs ----
    fw_tab0 = nc.dram_tensor("fw_tab0", [N, D], NDT, kind="ExternalInput")
    bw_tab0 = nc.dram_tensor("bw_tab0", [N, D], NDT, kind="ExternalInput")
    e_tab0 = nc.dram_tensor("e_tab0", [E, D], EDT, kind="ExternalInput")
    fw_own0 = nc.dram_tensor("fw_own0", [NS, D], BF16, kind="ExternalInput")
    bw_own0 = nc.dram_tensor("bw_own0", [NS, D], BF16, kind="ExternalInput")
    e_own0 = nc.dram_tensor("e_own0", [ES, D], BF16, kind="ExternalInput")

    idx_in = {}
    seg_in = {}
    for s in ("fa", "fe", "ba", "be", "df", "db"):
        tot = off[s][-1]
        idx_in[s] = nc.dram_tensor(f"idx_{s}", [P, tot * 8], I16,
                                   kind="ExternalInput")
        seg_in[s] = nc.dram_tensor(f"seg_{s}", [P, tot], F32,
                                   kind="ExternalInput")
    rcn_fw = nc.dram_tensor("rcn_fw", [P, NB], F32, kind="ExternalInput")
    rcn_bw = nc.dram_tensor("rcn_bw", [P, NB], F32, kind="ExternalInput")
    rce_fw = nc.dram_tensor("rce_fw", [P, EB], F32, kind="ExternalInput")
    rce_bw = nc.dram_tensor("rce_bw", [P, EB], F32, kind="ExternalInput")

    Wfc = nc.dram_tensor("Wfc", [2 * D, D], BF16, kind="ExternalInput")
    Wbc = nc.dram_tensor("Wbc", [2 * D, D], BF16, kind="ExternalInput")
    Wedge = nc.dram_tensor("Wedge", [3 * D, D], BF16, kind="ExternalInput")
    bfc = nc.dram_tensor("bfc", [1, D], BF16, kind="ExternalInput")
    bbc = nc.dram_tensor("bbc", [1, D], BF16, kind="ExternalInput")
    bedge = nc.dram_tensor("bedge", [1, D], BF16, kind="ExternalInput")
    fw_out = nc.dram_tensor("fw_out", [NS, D], F32, kind="ExternalOutput")
    bw_out = nc.dram_tensor("bw_out", [NS, D], F32, kind="ExternalOutput")

    rg = [list(range(CORES))]
    COPY = mybir.ActivationFunctionType.Copy
    RELU = mybir.ActivationFunctionType.Relu
    ISEQ = mybir.AluOpType.is_equal

    with tile.TileContext(nc) as tc:
        with (
            tc.tile_pool(name="const", bufs=1) as cp,
            tc.tile_pool(name="gap", bufs=7) as gap,
            tc.tile_pool(name="gep", bufs=4) as gep,
            tc.tile_pool(name="mp", bufs=28) as mp,
            tc.tile_pool(name="sp", bufs=8) as sp,
            tc.tile_pool(name="xp", bufs=4) as xp,
            tc.tile_pool(name="fhp", bufs=8) as fhp,
            tc.tile_pool(name="pm", bufs=3, space="PSUM") as pmp,
            tc.tile_pool(name="pt", bufs=2, space="PSUM") as ptp,
            tc.tile_pool(name="po", bufs=2, space="PSUM") as pop,
            tc.tile_pool(name="dram", bufs=1, space="DRAM") as dp,
        ):
            # ---- constants ----
            ident = cp.tile([P, P], BF16)
            make_identity(nc, ident[:])
            iota_t = cp.tile([P, P], BF16, name="iota_t")
            nc.gpsimd.iota(iota_t[:], pattern=[[1, P]], base=0,
                           channel_multiplier=0,
                           allow_small_or_imprecise_dtypes=True)
            ones1 = cp.tile([1, P], BF16, name="ones1")
            nc.gpsimd.memset(ones1[:], 1.0)

            def load_w(name, src, kc):
                t = cp.tile([P, kc * D], BF16, name=name)
                for kk in range(kc):
                    nc.sync.dma_start(out=t[:, kk * D:(kk + 1) * D],
                                      in_=src[kk * P:(kk + 1) * P, :])
                return t

            wfc_t = load_w("wfc_t", Wfc, KCN)
            wbc_t = load_w("wbc_t", Wbc, KCN)
            we_t = load_w("we_t", Wedge, KCE)

            def load_flat(name, src, shape, dt):
                t = cp.tile(shape, dt, name=name)
                nc.sync.dma_start(out=t[:], in_=src[:])
                return t

            bfc_t = load_flat("bfc_t", bfc, [1, D], BF16)
            bbc_t = load_flat("bbc_t", bbc, [1, D], BF16)
            be_t = load_flat("be_t", bedge, [1, D], BF16)

            idx_sb = {s: load_flat(f"ti_{s}", t, [P, t.shape[1]], I16)
                      for s, t in idx_in.items()}
            seg_sb = {s: load_flat(f"ts_{s}", t, [P, t.shape[1]], F32)
                      for s, t in seg_in.items()}
            rc_sb = {nm: load_flat(f"tr_{nm}", t, [P, t.shape[1]], F32)
                     for nm, t in (("rcn_fw", rcn_fw), ("rcn_bw", rcn_bw),
                                   ("rce_fw", rce_fw), ("rce_bw", rce_bw))}

            # ---- DRAM tables (internal, chunk-permuted row layout) ----
            def mk_tab(name, rows, dt):
                return dp.tile([rows, D], dt, addr_space="Shared", name=name)

            fw_ntabs = [fw_tab0, mk_tab("fw_tabA", N, NDT),
                        mk_tab("fw_tabB", N, NDT)]
            bw_ntabs = [bw_tab0, mk_tab("bw_tabA", N, NDT),
                        mk_tab("bw_tabB", N, NDT)]
            e_tabs = [e_tab0, mk_tab("e_tabA", E, EDT),
                      mk_tab("e_tabB", E, EDT)]
            fw_shB = dp.tile([NS, D], BF16, name="fw_shB")
            bw_shB = dp.tile([NS, D], BF16, name="bw_shB")
            e_shB = dp.tile([ES, D], BF16, name="e_shB")
            fw_sh8 = (dp.tile([NS, D], NDT, name="fw_sh8")
                      if nt8 else fw_shB)
            bw_sh8 = (dp.tile([NS, D], NDT, name="bw_sh8")
                      if nt8 else bw_shB)
            e_sh8 = (dp.tile([ES, D], EDT, name="e_sh8")
                     if et8 else e_shB)

            # >1024 indices in one dma_gather crashes on HW (descriptor ring
            # capacity); split calls into <=8 tile (1024 idx) chunks.
            gchunk = int(_os.environ.get("GNN_GCHUNK", "8"))

            def gather_q(tab, s, b, pool, tag, dt):
                T = tiles[s][b]
                g = pool.tile([P, T * D], dt, name=f"g_{tag}", tag=tag)
                ch = gchunk if gchunk > 0 else T
                for j0 in range(0, T, ch):
                    ct = min(ch, T - j0)
                    nc.gpsimd.dma_gather(
                        out_ap=g[:, j0 * D:(j0 + ct) * D].rearrange(
                            "p (t e) -> p t e", e=D),
                        in_ap=tab[:],
                        idxs_ap=idx_sb[s][:, (off[s][b] + j0) * 8:
                                          (off[s][b] + j0 + ct) * 8],
                        num_idxs=ct * P,
                        num_idxs_reg=ct * P,
                        elem_size=D,
                        queue_num=0,  # rewritten post-scheduling below
                    )
                return g

            def mean_matmuls(pm, g, s, b, start, stop):
                """Accumulate segment-sums of gather tile g into psum pm."""
                T = tiles[s][b]
                o = off[s][b]
                for t in range(T):
                    M = mp.tile([P, P], g.tensor.dtype, name="M", tag="M")
                    nc.vector.tensor_scalar(
                        out=M[:], in0=iota_t[:],
                        scalar1=seg_sb[s][:, o + t:o + t + 1],
                        scalar2=None, op0=ISEQ)
                    nc.tensor.matmul(
                        out=pm[:], lhsT=M[:], rhs=g[:, t * D:(t + 1) * D],
                        start=(start and t == 0), stop=(stop and t == T - 1))

            def transpose_to(xT, cbase, src_sb, eng):
                """PE-transpose [P, D] src into xT chunks, copy via eng."""
                pt = ptp.tile([P, DC * P], BF16, name="ps_t")
                for c in range(DC):
                    nc.tensor.transpose(
                        out=pt[:, c * P:(c + 1) * P],
                        in_=src_sb[:, c * P:(c + 1) * P],
                        identity=ident[:],
                    )
                if eng == "s":
                    nc.scalar.activation(
                        out=xT[:, cbase * P:(cbase + DC) * P], in_=pt[:],
                        func=COPY)
                else:
                    nc.vector.tensor_copy(
                        out=xT[:, cbase * P:(cbase + DC) * P], in_=pt[:])

            use_xbar = _os.environ.get("GNN_XBAR", "1") == "1"

            def load_own_T(xT, own_src, b):
                """Own rows into xT[:, 0:D] transposed."""
                if use_xbar:
                    nc.sync.dma_start(
                        out=xT[:, 0:D].rearrange("p (c q) -> p c q", q=P),
                        in_=own_src[b * P:(b + 1) * P, :],
                        transpose=True)
                else:
                    nf = sp.tile([P, D], BF16, name="nf", tag="sm")
                    nc.sync.dma_start(out=nf[:],
                                      in_=own_src[b * P:(b + 1) * P, :])
                    transpose_to(xT, 0, nf, "s")

            def linear(xT, kc, w_t, b_row, relu, out_sb):
                po = pop.tile([P, D], F32, name="ps_o")
                for kk in range(kc):
                    nc.tensor.matmul(
                        out=po[:], lhsT=xT[:, kk * P:(kk + 1) * P],
                        rhs=w_t[:, kk * D:(kk + 1) * D],
                        start=(kk == 0), stop=False,
                    )
                nc.tensor.matmul(
                    out=po[:], lhsT=ones1[:], rhs=b_row[:], start=False,
                    stop=True,
                )
                nc.scalar.activation(out=out_sb[:], in_=po[:],
                                     func=(RELU if relu else COPY))
                return po

            def node_block(k, b, ga, ge, own_src, sa, se, rc_nm,
                           w_t, b_row, dst, dst8, last):
                pm = pmp.tile([P, D], F32, name="pm", tag="pm")
                mean_matmuls(pm, ga, sa, b, start=True, stop=False)
                mean_matmuls(pm, ge, se, b, start=False, stop=True)
                sm = sp.tile([P, D], BF16, name="sm", tag="sm")
                nc.scalar.activation(out=sm[:], in_=pm[:], func=COPY,
                                     scale=rc_sb[rc_nm][:, b:b + 1])

                xT = xp.tile([P, KCN * P], BF16, name="xT", tag="xT")
                load_own_T(xT, own_src, b)
                transpose_to(xT, DC, sm, "v")

                fh = fhp.tile([P, D], F32 if last else BF16, name="fh",
                              tag="fh")
                po = linear(xT, KCN, w_t, b_row, not last, fh)
                nc.sync.dma_start(out=dst[b * P:(b + 1) * P, :], in_=fh[:])
                if not last and nt8:
                    fh8 = fhp.tile([P, D], NDT, name="fh8", tag="fh8")
                    nc.scalar.activation(out=fh8[:], in_=po[:], func=RELU)
                    nc.sync.dma_start(out=dst8[b * P:(b + 1) * P, :],
                                      in_=fh8[:])

            def edge_block(b, gf, gb, own_src):
                pf = pmp.tile([P, D], F32, name="pf", tag="pm")
                mean_matmuls(pf, gf, "df", b, start=True, stop=True)
                smf = sp.tile([P, D], BF16, name="smf", tag="sm")
                nc.scalar.activation(out=smf[:], in_=pf[:], func=COPY,
                                     scale=rc_sb["rce_fw"][:, b:b + 1])

                pb = pmp.tile([P, D], F32, name="pb", tag="pm")
                mean_matmuls(pb, gb, "db", b, start=True, stop=True)
                smb = sp.tile([P, D], BF16, name="smb", tag="sm")
                nc.scalar.activation(out=smb[:], in_=pb[:], func=COPY,
                                     scale=rc_sb["rce_bw"][:, b:b + 1])

                xT = xp.tile([P, KCE * P], BF16, name="xTe", tag="xT")
                load_own_T(xT, own_src, b)
                transpose_to(xT, DC, smf, "s")
                transpose_to(xT, 2 * DC, smb, "v")

                es = fhp.tile([P, D], BF16, name="es", tag="fh")
                po = linear(xT, KCE, we_t, be_t, True, es)
                nc.sync.dma_start(out=e_shB[b * P:(b + 1) * P, :], in_=es[:])
                if et8:
                    es8 = fhp.tile([P, D], EDT, name="es8", tag="fh8")
                    nc.scalar.activation(out=es8[:], in_=po[:], func=RELU)
                    nc.sync.dma_start(out=e_sh8[b * P:(b + 1) * P, :],
                                      in_=es8[:])

            def allgather_chunk(src_sh, dsttab, i, cr):
                nc.gpsimd.collective_compute(
                    "AllGather", mybir.AluOpType.bypass, replica_groups=rg,
                    ins=[src_sh[i * cr:(i + 1) * cr, :]],
                    outs=[dsttab[i * CORES * cr:(i + 1) * CORES * cr, :]],
                )

            crN = NS // NCH_N
            crE = ES // NCH_E
            nbc = NB // NCH_N   # node blocks per AG chunk
            ebc = EB // NCH_E
            PRE = int(_os.environ.get("GNN_PRE", "6"))

            def node_phase(k, ntab, etab, own_src, sa, se, rc_nm, w_t, b_row,
                           dst, dst8, sh8, dsttab, last):
                # prefetch the primary-stream gathers so their transfers can
                # overlap the AllGather / eid-table wait at phase entry
                gaq = {b: gather_q(ntab, sa, b, gap, "ga", NDT)
                       for b in range(min(PRE, NB))}
                geq = {}
                for b in range(NB):
                    if b not in geq:
                        geq[b] = gather_q(etab, se, b, gep, "ge", EDT)
                    if b + PRE < NB:
                        gaq[b + PRE] = gather_q(ntab, sa, b + PRE, gap,
                                                "ga", NDT)
                    if b + 1 < NB and b + 1 not in geq:
                        geq[b + 1] = gather_q(etab, se, b + 1, gep, "ge", EDT)
                    node_block(k, b, gaq.pop(b), geq.pop(b), own_src,
                               sa, se, rc_nm, w_t, b_row, dst, dst8, last)
                    if not last and (b + 1) % nbc == 0:
                        allgather_chunk(sh8, dsttab, (b + 1) // nbc - 1, crN)

            def edge_phase(k, fw_nt, bw_nt, own_src):
                gfq = {b: gather_q(fw_nt, "df", b, gap, "ga", NDT)
                       for b in range(min(PRE, EB))}
                gbq = {}
                for b in range(EB):
                    if b not in gbq:
                        gbq[b] = gather_q(bw_nt, "db", b, gep, "ge", NDT)
                    if b + PRE < EB:
                        gfq[b + PRE] = gather_q(fw_nt, "df", b + PRE, gap,
                                                "ga", NDT)
                    if b + 1 < EB and b + 1 not in gbq:
                        gbq[b + 1] = gather_q(bw_nt, "db", b + 1, gep,
                                              "ge", NDT)
                    edge_block(b, gfq.pop(b), gbq.pop(b), own_src)
                    if (b + 1) % ebc == 0:
                        allgather_chunk(e_sh8, e_tabs[k + 1],
                                        (b + 1) // ebc - 1, crE)

            for k in range(K):
                last = (k == K - 1)
                node_phase(k, fw_ntabs[k], e_tabs[k],
                           fw_own0 if k == 0 else fw_shB,
                           "fa", "fe", "rcn_fw", wfc_t, bfc_t,
                           fw_out if last else fw_shB, fw_sh8, fw_sh8,
                           None if last else fw_ntabs[k + 1], last)
                node_phase(k, bw_ntabs[k], e_tabs[k],
                           bw_own0 if k == 0 else bw_shB,
                           "ba", "be", "rcn_bw", wbc_t, bbc_t,
                           bw_out if last else bw_shB, bw_sh8, bw_sh8,
                           None if last else bw_ntabs[k + 1], last)
                if not last:
                    edge_phase(k, fw_ntabs[k + 1], bw_ntabs[k + 1],
                               e_own0 if k == 0 else e_shB)

    # The Tile scheduler assigns DMASW sem lanes to SWDGE DMA instructions
    # by final program order (mod 8), and each sem lane must stick to one
    # SWDGE queue. Rewrite queue_num post-scheduling to match: lane i ->
    # queue i%4 (4 divides 8, so each lane sees a single queue).
    if _os.environ.get("GNN_Q0", "0") != "1":
        i = 0
        for bb in nc.m.functions[0].blocks:
            for inst in bb.instructions:
                if isinstance(inst, mybir.InstDMAGatherAnt):
                    inst.queue_num = i % 4
                    i += 1

    nc.compile()
    return nc


def _chunk_perm(n_total, shard, cr, cores):
    """Row permutation so AllGather chunks write contiguous table ranges:
    node n (core c, local r, chunk i, offset j) -> i*cores*cr + c*cr + j."""
    n = np.arange(n_total)
    c, r = n // shard, n % shard
    i, j = r // cr, r % cr
    return i * (cores * cr) + c * cr + j


def _pack_stream(idx_sh, nblk, Tb, perm):
    """idx_sh [nblk*128, W] int with -1 pads -> (idx16 [128, sum(Tb)*8],
    seg [128, sum(Tb)]) packed real-first per block, row-0/-1 padding.
    Real indices are remapped through perm (chunk-permuted table layout)."""
    m = idx_sh >= 0
    icols, scols = [], []
    for b in range(nblk):
        blk = idx_sh[b * P:(b + 1) * P]
        r, c = np.nonzero(m[b * P:(b + 1) * P])
        vals = perm[blk[r, c]]
        tot = Tb[b] * P
        assert len(vals) <= tot
        vi = np.zeros(tot, np.int64)
        sg = np.full(tot, -1.0, np.float32)
        vi[:len(vals)] = vals
        sg[:len(vals)] = r
        icols.append(np.tile(vi.reshape(-1, 16).T, (8, 1)).astype(np.int16))
        scols.append(np.ascontiguousarray(sg.reshape(Tb[b], P).T))
    return (np.ascontiguousarray(np.concatenate(icols, 1)),
            np.ascontiguousarray(
                np.concatenate(scols, 1).astype(np.float32)))


def prep_inputs(cfg: Cfg, inputs: dict, nt8=True, et8=True):
    import ml_dtypes
    N, E, D, CORES = cfg.N, cfg.E, cfg.D, cfg.CORES
    NS, ES, NB, EB = cfg.NS, cfg.ES, cfg.NB, cfg.EB
    f32 = np.float32
    bt = ml_dtypes.bfloat16
    ndt = ml_dtypes.float8_e4m3 if nt8 else bt
    edt = ml_dtypes.float8_e4m3 if et8 else bt

    fw = np.ascontiguousarray(np.asarray(inputs["fw_input"], f32))
    bw = np.ascontiguousarray(np.asarray(inputs["bw_input"], f32))
    ee = np.ascontiguousarray(np.asarray(inputs["edge_embs"], f32))

    perm_n = _chunk_perm(N, NS, NS // cfg.NCH_N, CORES)
    perm_e = _chunk_perm(E, ES, ES // cfg.NCH_E, CORES)

    def permuted(x, perm):
        out = np.empty_like(x)
        out[perm] = x
        return out

    fw_tab = permuted(fw, perm_n).astype(ndt)
    bw_tab = permuted(bw, perm_n).astype(ndt)
    ee_tab = permuted(ee, perm_e).astype(edt)

    idxs = {k: np.asarray(inputs[k], np.int64) for k in
            ("fw_adj", "bw_adj", "fw_edgeid", "bw_edgeid",
             "fw_edgedep", "bw_edgedep")}
    deg = {k: (v >= 0).sum(1) for k, v in idxs.items()}

    # per-(core, block-slot) real row counts -> per-slot tile counts
    strm = {"fa": ("fw_adj", NB, NS, perm_n), "fe": ("fw_edgeid", NB, NS, perm_e),
            "ba": ("bw_adj", NB, NS, perm_n), "be": ("bw_edgeid", NB, NS, perm_e),
            "df": ("fw_edgedep", EB, ES, perm_n),
            "db": ("bw_edgedep", EB, ES, perm_n)}
    tiles = {}
    for s, (k, nblk, sh, _) in strm.items():
        cnt = deg[k].reshape(CORES, nblk, P).sum(2)    # [CORES, nblk]
        mx = cnt.max(0)                                # per block slot
        tiles[s] = [int(-(-v // P)) for v in mx]
        assert all(t >= 1 for t in tiles[s])

    def rec(c):
        return (1.0 / np.maximum(c, 1).astype(f32)).astype(f32)

    rcn_fw_f = rec(deg["fw_adj"] + deg["fw_edgeid"])
    rcn_bw_f = rec(deg["bw_adj"] + deg["bw_edgeid"])
    rce_fw_f = rec(deg["fw_edgedep"])
    rce_bw_f = rec(deg["bw_edgedep"])

    Wfc = np.asarray(inputs["Wfc"], f32).astype(bt)
    Wbc = np.asarray(inputs["Wbc"], f32).astype(bt)
    Wedge = np.asarray(inputs["Wedge"], f32).astype(bt)
    bfc = np.asarray(inputs["bfc"], f32).reshape(1, D).astype(bt)
    bbc = np.asarray(inputs["bbc"], f32).reshape(1, D).astype(bt)
    bedge = np.asarray(inputs["bedge"], f32).reshape(1, D).astype(bt)

    in_maps = []
    for c in range(CORES):
        nsl = slice(c * NS, (c + 1) * NS)
        esl = slice(c * ES, (c + 1) * ES)
        im = {
            "fw_tab0": fw_tab, "bw_tab0": bw_tab, "e_tab0": ee_tab,
            "fw_own0": fw[nsl].astype(bt), "bw_own0": bw[nsl].astype(bt),
            "e_own0": ee[esl].astype(bt),
            "rcn_fw": np.ascontiguousarray(rcn_fw_f[nsl].reshape(NB, P).T),
            "rcn_bw": np.ascontiguousarray(rcn_bw_f[nsl].reshape(NB, P).T),
            "rce_fw": np.ascontiguousarray(rce_fw_f[esl].reshape(EB, P).T),
            "rce_bw": np.ascontiguousarray(rce_bw_f[esl].reshape(EB, P).T),
            "Wfc": Wfc, "Wbc": Wbc, "Wedge": Wedge,
            "bfc": bfc, "bbc": bbc, "bedge": bedge,
        }
        for s, (k, nblk, sh, perm) in strm.items():
            sl = nsl if sh == NS else esl
            i16, sg = _pack_stream(idxs[k][sl], nblk, tiles[s], perm)
            im[f"idx_{s}"] = i16
            im[f"seg_{s}"] = sg
        in_maps.append(im)
    return in_maps, tiles


def assemble_outputs(cfg: Cfg, results):
    fw = np.concatenate([results[c]["fw_out"] for c in range(cfg.CORES)],
                        axis=0)
    bw = np.concatenate([results[c]["bw_out"] for c in range(cfg.CORES)],
                        axis=0)
    return fw, bw


# ======================= self-contained runner =======================
import types as _types


def _install_axon_prof():
    """Provide antenv.axon_hooks + NTFF hook so trace=True works under axon."""
    name = "antenv.axon_hooks"
    if name in sys.modules:
        return True
    try:
        mod = _types.ModuleType(name)
        mod._hook = None
        mod.set_axon_ntff_profile_hook = lambda h: setattr(mod, "_hook", h)
        mod.get_axon_ntff_profile_hook = lambda: mod._hook
        sys.modules[name] = mod
        import antenv
        antenv.axon_hooks = mod
        from trn_agent_boot.trn_boot import _ntff_profile_via_ctypes
        mod.set_axon_ntff_profile_hook(
            _ntff_profile_via_ctypes('/opt/axon/libaxon_pjrt.so'))
        return True
    except Exception:
        sys.modules.pop(name, None)
        return False


_CACHE = {}
LAST_EXEC_NS = None
LAST_PROFILE = None


def kernel(**inputs):
    """Full-input GNN forward on 8 TRN2 NeuronCores. Returns (fw, bw)."""
    global LAST_EXEC_NS, LAST_PROFILE
    from concourse.bass_utils import run_bass_kernel_spmd

    cfg = Cfg()
    nt8 = _os.environ.get("GNN_NT8", "1") == "1"
    et8 = _os.environ.get("GNN_ET8", "1") == "1"
    in_maps, tiles = prep_inputs(cfg, inputs, nt8=nt8, et8=et8)
    key = (nt8, et8, tuple((s, tuple(ts)) for s, ts in sorted(tiles.items())))
    if _CACHE.get("key") != key:
        _CACHE["nc"] = build(cfg, tiles, nt8=nt8, et8=et8)
        _CACHE["key"] = key
    nc = _CACHE["nc"]

    profile = _os.environ.get("GNN_PROFILE", "0") == "1"
    if profile:
        profile = _install_axon_prof()
    res = run_bass_kernel_spmd(nc, in_maps, core_ids=list(range(cfg.CORES)),
                               trace=profile)
    LAST_EXEC_NS = res.exec_time_ns
    LAST_PROFILE = res.profile_json
    if res.instructions_and_trace is not None:
        try:
            print("trace:", res.instructions_and_trace[1])
        except Exception:
            pass
    return assemble_outputs(cfg, res.results)
